# revision 14
# baseline (speedup 1.0000x reference)
"""Trainium2 Bass kernel for an Elman RNN (nn_BaselineRNN).

h_t = tanh(x_t @ Wx + h_{t-1} @ Wh + b_i2h); output o = h_S @ W_h2o.T + b_h2o
for the last step only. Returns (output (B,1,O), hidden (B,H)).

Sharding: data-parallel over the batch dim across 8 NeuronCores (16 rows
per core); weights replicated. The sequential scan runs locally per core.

Per-core layout: the hidden state lives transposed ("hT": H on the 128-way
partition dim as 4 chunks of (128, 16-batch)). The per-step update
    z_m = sum_k Wh[k,m].T @ hT_k  (+ px_t + b, then tanh)
takes hT chunks in and produces hT chunks out, so the scan needs no
per-step transpose. x @ Wx is precomputed in blocks of 128 steps with
N=512 float32r matmuls (full-speed PE) into SBUF; the per-step tanh+bias
is a single ScalarE activation per chunk reading PSUM.
"""

import sys

sys.path.insert(0, "/opt/trn_rl_repo")

import numpy as np

import concourse.bacc as bacc
import concourse.bass as bass
import concourse.mybir as mybir
import concourse.tile as tile
from concourse.bass_utils import run_bass_kernel_spmd
from concourse.masks import make_identity

N_CORES = 8
B, S, I, H, O = 128, 1024, 256, 512, 256
BL = B // N_CORES  # 16 rows per core
F32 = mybir.dt.float32
F32R = mybir.dt.float32r

BLK = 128  # time steps per pipeline block


def _r(ap):
    return ap.bitcast(F32R)


def build_nc(n_steps=S, recur_dtype="fp32", dbg=False):
    """Build + compile the per-core Bass module. recur_dtype: fp32 | fp32r."""
    assert n_steps % BLK == 0
    nblk = n_steps // BLK
    nc = bacc.Bacc("TRN2", target_bir_lowering=False, debug=False,
                   enable_asserts=True, num_devices=N_CORES)
    if dbg:
        xt_d = nc.dram_tensor("xt_dump", [128, I // 128, BLK * BL], F32,
                              kind="ExternalOutput").ap()
        px_d = nc.dram_tensor("px_dump", [128, H // 128, BLK * BL], F32,
                              kind="ExternalOutput").ap()
        h1_d = nc.dram_tensor("h1_dump", [128, H // 128, BL], F32,
                              kind="ExternalOutput").ap()

    seq = nc.dram_tensor("sequences", [BL, S, I], F32, kind="ExternalInput").ap()
    w_i2h = nc.dram_tensor("W_i2h", [H, I + H], F32, kind="ExternalInput").ap()
    b_i2h = nc.dram_tensor("b_i2h", [H], F32, kind="ExternalInput").ap()
    w_h2o = nc.dram_tensor("W_h2o", [O, H], F32, kind="ExternalInput").ap()
    b_h2o = nc.dram_tensor("b_h2o", [O], F32, kind="ExternalInput").ap()
    out_d = nc.dram_tensor("output", [BL, 1, O], F32, kind="ExternalOutput").ap()
    hid_d = nc.dram_tensor("hidden", [BL, H], F32, kind="ExternalOutput").ap()

    KI = I // 128   # 2 contraction chunks for x@Wx
    KH = H // 128   # 4 contraction chunks for h@Wh
    MH = H // 128   # 4 output chunks of H
    MO = O // 128   # 2 output chunks of O

    with tile.TileContext(nc) as tc:
        with (
            tc.tile_pool(name="const", bufs=1) as cpool,
            tc.tile_pool(name="xnat", bufs=4) as xnat_pool,
            tc.tile_pool(name="xt", bufs=2) as xt_pool,
            tc.tile_pool(name="px", bufs=2) as px_pool,
            tc.tile_pool(name="h", bufs=2) as h_pool,
            tc.tile_pool(name="ztmp", bufs=8) as z_pool,
            tc.tile_pool(name="tp_ps", bufs=2, space="PSUM") as tp_psum,
            tc.tile_pool(name="px_ps", bufs=2, space="PSUM") as px_psum,
            tc.tile_pool(name="z_ps", bufs=4, space="PSUM") as z_psum,
        ):
            ident = cpool.tile([128, 128], F32)
            make_identity(nc, ident[:])

            # --- load weights (natural layout) and pre-transpose on PE ---
            w_nat = cpool.tile([128, 4, I + H], F32)
            nc.sync.dma_start(w_nat[:], w_i2h.rearrange("(a p) c -> p a c", p=128))
            wo_nat = cpool.tile([128, 2, H], F32)
            nc.sync.dma_start(wo_nat[:], w_h2o.rearrange("(a p) c -> p a c", p=128))

            # Wx chunks: wx[:, kc, mh*128+f] = W_i2h[mh*128+f, kc*128+p]
            # float32r: walrus requires fp32r matmul operands to be written
            # by an instruction that rounds to fp32r, so the tiles are
            # natively fp32r and the DVE copies do the rounding.
            wx = cpool.tile([128, KI, H], F32R)
            for kc in range(KI):
                for mh in range(MH):
                    ps = tp_psum.tile([128, 128], F32)
                    nc.tensor.transpose(
                        ps[:], w_nat[:, mh, kc * 128:(kc + 1) * 128], ident[:])
                    nc.vector.tensor_copy(wx[:, kc, mh * 128:(mh + 1) * 128], ps[:])

            # Wh chunks: wh[:, kc, mh*128+f] = W_i2h[mh*128+f, I + kc*128+p]
            wh = cpool.tile([128, KH, H], F32)
            for kc in range(KH):
                for mh in range(MH):
                    ps = tp_psum.tile([128, 128], F32)
                    nc.tensor.transpose(
                        ps[:], w_nat[:, mh, I + kc * 128:I + (kc + 1) * 128], ident[:])
                    nc.vector.tensor_copy(wh[:, kc, mh * 128:(mh + 1) * 128], ps[:])

            # W_h2o.T chunks: wo[:, kc, mo*128+f] = W_h2o[mo*128+f, kc*128+p]
            wo = cpool.tile([128, KH, O], F32)
            for kc in range(KH):
                for mo in range(MO):
                    ps = tp_psum.tile([128, 128], F32)
                    nc.tensor.transpose(
                        ps[:], wo_nat[:, mo, kc * 128:(kc + 1) * 128], ident[:])
                    nc.vector.tensor_copy(wo[:, kc, mo * 128:(mo + 1) * 128], ps[:])

            # biases: per-partition columns
            bi = cpool.tile([128, MH], F32)
            nc.sync.dma_start(bi[:], b_i2h.rearrange("(m p) -> p m", p=128))
            bo = cpool.tile([128, MO], F32)
            nc.sync.dma_start(bo[:], b_h2o.rearrange("(m p) -> p m", p=128))

            h_prev = h_pool.tile([128, KH, BL], F32)
            nc.vector.memset(h_prev[:], 0.0)

            cast = _r if recur_dtype == "fp32r" else (lambda ap: ap)

            for blk in range(nblk):
                # --- xT for this block: rows ordered batch-major
                # (row = b*BLK + s), so each x load is one contiguous
                # (128 steps, 256) tile for a single batch element.
                xt = xt_pool.tile([128, KI, BL * BLK], F32R)
                for bb in range(BL):
                    xn = xnat_pool.tile([128, I], F32)
                    nc.sync.dma_start(
                        xn[:], seq[bb, blk * BLK:(blk + 1) * BLK, :])
                    for kc in range(KI):
                        ps = tp_psum.tile([128, 128], F32)
                        nc.tensor.transpose(
                            ps[:], xn[:, kc * 128:(kc + 1) * 128], ident[:])
                        nc.vector.tensor_copy(
                            xt[:, kc, bb * BLK:(bb + 1) * BLK], ps[:])

                # --- px = x @ Wx for this block (float32r, N=512) ---
                # free order of px is (b, s): per-step slice is stride-BLK
                px = px_pool.tile([128, MH, BL * BLK], F32)
                px_bs = px.rearrange("p m (b s) -> p m b s", s=BLK)
                for rg in range(BL * BLK // 512):
                    for mh in range(MH):
                        pps = px_psum.tile([128, 512], F32)
                        for kc in range(KI):
                            nc.tensor.matmul(
                                pps[:],
                                wx[:, kc, mh * 128:(mh + 1) * 128],
                                xt[:, kc, rg * 512:(rg + 1) * 512],
                                start=(kc == 0), stop=(kc == KI - 1))
                        nc.vector.tensor_copy(
                            px[:, mh, rg * 512:(rg + 1) * 512], pps[:])

                if dbg and blk == 0:
                    nc.sync.dma_start(xt_d, xt[:].bitcast(F32))
                    nc.sync.dma_start(px_d, px[:])

                # --- the sequential scan ---
                for j in range(BLK):
                    h_new = h_pool.tile([128, KH, BL], F32)
                    for mh in range(MH):
                        zps = z_psum.tile([128, BL], F32)
                        for kc in range(KH):
                            nc.tensor.matmul(
                                zps[:],
                                cast(wh[:, kc, mh * 128:(mh + 1) * 128]),
                                cast(h_prev[:, kc, :]),
                                start=(kc == 0), stop=(kc == KH - 1))
                        zt = z_pool.tile([128, BL], F32)
                        nc.vector.tensor_add(
                            zt[:], zps[:], px_bs[:, mh, :, j])
                        nc.scalar.activation(
                            h_new[:, mh, :], zt[:],
                            mybir.ActivationFunctionType.Tanh,
                            bias=bi[:, mh:mh + 1])
                    h_prev = h_new
                    if dbg and blk == 0 and j == 0:
                        nc.sync.dma_start(h1_d, h_prev[:])

            # --- output projection from the final hidden state ---
            for mo in range(MO):
                ops = z_psum.tile([128, BL], F32, tag="zps")
                for kc in range(KH):
                    nc.tensor.matmul(
                        ops[:],
                        cast(wo[:, kc, mo * 128:(mo + 1) * 128]),
                        cast(h_prev[:, kc, :]),
                        start=(kc == 0), stop=(kc == KH - 1))
                o_sb = z_pool.tile([128, BL], F32, tag="zt")
                nc.vector.tensor_scalar_add(o_sb[:], ops[:], bo[:, mo:mo + 1])
                nc.sync.dma_start(
                    out_d.rearrange("b one (m p) -> m p (b one)", p=128)[mo],
                    o_sb[:])

            for kc in range(KH):
                nc.sync.dma_start(
                    hid_d.rearrange("b (k p) -> k p b", p=128)[kc],
                    h_prev[:, kc, :])

    nc.compile()
    return nc


_NC_CACHE = {}


def get_nc(n_steps=S, recur_dtype="fp32"):
    key = (n_steps, recur_dtype)
    if key not in _NC_CACHE:
        _NC_CACHE[key] = build_nc(n_steps, recur_dtype)
    return _NC_CACHE[key]


def make_in_maps(sequences, W_i2h, b_i2h, W_h2o, b_h2o):
    sequences = np.ascontiguousarray(np.asarray(sequences, dtype=np.float32))
    W_i2h = np.ascontiguousarray(np.asarray(W_i2h, dtype=np.float32))
    b_i2h = np.ascontiguousarray(np.asarray(b_i2h, dtype=np.float32))
    W_h2o = np.ascontiguousarray(np.asarray(W_h2o, dtype=np.float32))
    b_h2o = np.ascontiguousarray(np.asarray(b_h2o, dtype=np.float32))
    return [
        {
            "sequences": sequences[c * BL:(c + 1) * BL],
            "W_i2h": W_i2h,
            "b_i2h": b_i2h,
            "W_h2o": W_h2o,
            "b_h2o": b_h2o,
        }
        for c in range(N_CORES)
    ]


def kernel(sequences, W_i2h, b_i2h, W_h2o, b_h2o):
    nc = get_nc()
    in_maps = make_in_maps(sequences, W_i2h, b_i2h, W_h2o, b_h2o)
    res = run_bass_kernel_spmd(nc, in_maps, list(range(N_CORES))).results
    output = np.concatenate([res[c]["output"] for c in range(N_CORES)], axis=0)
    hidden = np.concatenate([res[c]["hidden"] for c in range(N_CORES)], axis=0)
    return output, hidden


# revision 16
# speedup vs baseline: 11.3939x; 11.3939x over previous
"""Trainium2 Bass kernel for an Elman RNN (nn_BaselineRNN).

h_t = tanh(x_t @ Wx + h_{t-1} @ Wh + b_i2h); output o = h_S @ W_h2o.T + b_h2o
for the last step only. Returns (output (B,1,O), hidden (B,H)).

Sharding: data-parallel over the batch dim across 8 NeuronCores (16 rows
per core); weights replicated. The sequential scan runs locally per core.

Per-core layout: the hidden state lives transposed ("hT": H on the 128-way
partition dim as 4 chunks of (128, 16-batch)). The per-step update
    z_m = sum_k Wh[k,m].T @ hT_k  (+ px_t + b, then tanh)
takes hT chunks in and produces hT chunks out, so the scan needs no
per-step transpose. x @ Wx is precomputed in blocks of 128 steps with
N=512 float32r matmuls (full-speed PE) into SBUF; the per-step tanh+bias
is a single ScalarE activation per chunk reading PSUM.
"""

import sys

sys.path.insert(0, "/opt/trn_rl_repo")

import numpy as np

import concourse.bacc as bacc
import concourse.bass as bass
import concourse.mybir as mybir
import concourse.tile as tile
from concourse.bass_utils import run_bass_kernel_spmd
from concourse.masks import make_identity

N_CORES = 8
B, S, I, H, O = 128, 1024, 256, 512, 256
BL = B // N_CORES  # 16 rows per core
F32 = mybir.dt.float32
F32R = mybir.dt.float32r

BLK = 128  # time steps per pipeline block


def _r(ap):
    return ap.bitcast(F32R)


def build_nc(n_steps=S, recur_dtype="fp32", dbg=False, repeat=1):
    """Build + compile the per-core Bass module. recur_dtype: fp32 | fp32r.

    repeat > 1 re-runs the whole computation (for slope timing); the data
    is reused so numerics are meaningless beyond repeat 1, but the
    instruction stream per rep is identical.
    """
    assert n_steps % BLK == 0
    nblk = n_steps // BLK
    nc = bacc.Bacc("TRN2", target_bir_lowering=False, debug=False,
                   enable_asserts=True, num_devices=N_CORES)
    if dbg:
        xt_d = nc.dram_tensor("xt_dump", [128, I // 128, BLK * BL], F32,
                              kind="ExternalOutput").ap()
        px_d = nc.dram_tensor("px_dump", [128, H // 128, BLK * BL], F32,
                              kind="ExternalOutput").ap()
        h1_d = nc.dram_tensor("h1_dump", [128, H // 128, BL], F32,
                              kind="ExternalOutput").ap()

    seq = nc.dram_tensor("sequences", [BL, S, I], F32, kind="ExternalInput").ap()
    w_i2h = nc.dram_tensor("W_i2h", [H, I + H], F32, kind="ExternalInput").ap()
    b_i2h = nc.dram_tensor("b_i2h", [H], F32, kind="ExternalInput").ap()
    w_h2o = nc.dram_tensor("W_h2o", [O, H], F32, kind="ExternalInput").ap()
    b_h2o = nc.dram_tensor("b_h2o", [O], F32, kind="ExternalInput").ap()
    out_d = nc.dram_tensor("output", [BL, 1, O], F32, kind="ExternalOutput").ap()
    hid_d = nc.dram_tensor("hidden", [BL, H], F32, kind="ExternalOutput").ap()

    KI = I // 128   # 2 contraction chunks for x@Wx
    KH = H // 128   # 4 contraction chunks for h@Wh
    MH = H // 128   # 4 output chunks of H
    MO = O // 128   # 2 output chunks of O

    with tile.TileContext(nc) as tc:
        with (
            tc.tile_pool(name="const", bufs=1) as cpool,
            tc.tile_pool(name="xnat", bufs=4) as xnat_pool,
            tc.tile_pool(name="xt", bufs=2) as xt_pool,
            tc.tile_pool(name="px", bufs=2) as px_pool,
            tc.tile_pool(name="h", bufs=2) as h_pool,
            tc.tile_pool(name="ztmp", bufs=8) as z_pool,
            tc.tile_pool(name="tp_ps", bufs=2, space="PSUM") as tp_psum,
            tc.tile_pool(name="px_ps", bufs=2, space="PSUM") as px_psum,
            tc.tile_pool(name="z_ps", bufs=4, space="PSUM") as z_psum,
        ):
            ident = cpool.tile([128, 128], F32)
            make_identity(nc, ident[:])

            # --- load weights (natural layout) and pre-transpose on PE ---
            w_nat = cpool.tile([128, 4, I + H], F32)
            nc.sync.dma_start(w_nat[:], w_i2h.rearrange("(a p) c -> p a c", p=128))
            wo_nat = cpool.tile([128, 2, H], F32)
            nc.sync.dma_start(wo_nat[:], w_h2o.rearrange("(a p) c -> p a c", p=128))

            # Wx chunks: wx[:, kc, mh*128+f] = W_i2h[mh*128+f, kc*128+p]
            # float32r: walrus requires fp32r matmul operands to be written
            # by an instruction that rounds to fp32r, so the tiles are
            # natively fp32r and the DVE copies do the rounding.
            wx = cpool.tile([128, KI, H], F32R)
            for kc in range(KI):
                for mh in range(MH):
                    ps = tp_psum.tile([128, 128], F32)
                    nc.tensor.transpose(
                        ps[:], w_nat[:, mh, kc * 128:(kc + 1) * 128], ident[:])
                    nc.vector.tensor_copy(wx[:, kc, mh * 128:(mh + 1) * 128], ps[:])

            # Wh chunks: wh[:, kc, mh*128+f] = W_i2h[mh*128+f, I + kc*128+p]
            wh = cpool.tile([128, KH, H], F32)
            for kc in range(KH):
                for mh in range(MH):
                    ps = tp_psum.tile([128, 128], F32)
                    nc.tensor.transpose(
                        ps[:], w_nat[:, mh, I + kc * 128:I + (kc + 1) * 128], ident[:])
                    nc.vector.tensor_copy(wh[:, kc, mh * 128:(mh + 1) * 128], ps[:])

            # W_h2o.T chunks: wo[:, kc, mo*128+f] = W_h2o[mo*128+f, kc*128+p]
            wo = cpool.tile([128, KH, O], F32)
            for kc in range(KH):
                for mo in range(MO):
                    ps = tp_psum.tile([128, 128], F32)
                    nc.tensor.transpose(
                        ps[:], wo_nat[:, mo, kc * 128:(kc + 1) * 128], ident[:])
                    nc.vector.tensor_copy(wo[:, kc, mo * 128:(mo + 1) * 128], ps[:])

            # biases: per-partition columns
            bi = cpool.tile([128, MH], F32)
            nc.sync.dma_start(bi[:], b_i2h.rearrange("(m p) -> p m", p=128))
            bo = cpool.tile([128, MO], F32)
            nc.sync.dma_start(bo[:], b_h2o.rearrange("(m p) -> p m", p=128))

            h_prev = h_pool.tile([128, KH, BL], F32)
            nc.vector.memset(h_prev[:], 0.0)

            cast = _r if recur_dtype == "fp32r" else (lambda ap: ap)

            for gblk in range(nblk * repeat):
                blk = gblk % nblk
                # --- xT for this block: rows ordered batch-major
                # (row = b*BLK + s), so each x load is one contiguous
                # (128 steps, 256) tile for a single batch element.
                xt = xt_pool.tile([128, KI, BL * BLK], F32R)
                for bb in range(BL):
                    xn = xnat_pool.tile([128, I], F32)
                    nc.sync.dma_start(
                        xn[:], seq[bb, blk * BLK:(blk + 1) * BLK, :])
                    for kc in range(KI):
                        ps = tp_psum.tile([128, 128], F32)
                        nc.tensor.transpose(
                            ps[:], xn[:, kc * 128:(kc + 1) * 128], ident[:])
                        nc.vector.tensor_copy(
                            xt[:, kc, bb * BLK:(bb + 1) * BLK], ps[:])

                # --- px = x @ Wx for this block (float32r, N=512) ---
                # free order of px is (b, s): per-step slice is stride-BLK
                px = px_pool.tile([128, MH, BL * BLK], F32)
                px_bs = px.rearrange("p m (b s) -> p m b s", s=BLK)
                for rg in range(BL * BLK // 512):
                    for mh in range(MH):
                        pps = px_psum.tile([128, 512], F32)
                        for kc in range(KI):
                            nc.tensor.matmul(
                                pps[:],
                                wx[:, kc, mh * 128:(mh + 1) * 128],
                                xt[:, kc, rg * 512:(rg + 1) * 512],
                                start=(kc == 0), stop=(kc == KI - 1))
                        nc.vector.tensor_copy(
                            px[:, mh, rg * 512:(rg + 1) * 512], pps[:])

                if dbg and blk == 0:
                    nc.sync.dma_start(xt_d, xt[:].bitcast(F32))
                    nc.sync.dma_start(px_d, px[:])

                # --- the sequential scan ---
                for j in range(BLK):
                    h_new = h_pool.tile([128, KH, BL], F32)
                    for mh in range(MH):
                        zps = z_psum.tile([128, BL], F32)
                        for kc in range(KH):
                            nc.tensor.matmul(
                                zps[:],
                                cast(wh[:, kc, mh * 128:(mh + 1) * 128]),
                                cast(h_prev[:, kc, :]),
                                start=(kc == 0), stop=(kc == KH - 1))
                        zt = z_pool.tile([128, BL], F32)
                        nc.vector.tensor_add(
                            zt[:], zps[:], px_bs[:, mh, :, j])
                        nc.scalar.activation(
                            h_new[:, mh, :], zt[:],
                            mybir.ActivationFunctionType.Tanh,
                            bias=bi[:, mh:mh + 1])
                    h_prev = h_new
                    if dbg and blk == 0 and j == 0:
                        nc.sync.dma_start(h1_d, h_prev[:])

            # --- output projection from the final hidden state ---
            for mo in range(MO):
                ops = z_psum.tile([128, BL], F32, tag="zps")
                for kc in range(KH):
                    nc.tensor.matmul(
                        ops[:],
                        cast(wo[:, kc, mo * 128:(mo + 1) * 128]),
                        cast(h_prev[:, kc, :]),
                        start=(kc == 0), stop=(kc == KH - 1))
                o_sb = z_pool.tile([128, BL], F32, tag="zt")
                nc.vector.tensor_scalar_add(o_sb[:], ops[:], bo[:, mo:mo + 1])
                nc.sync.dma_start(
                    out_d.rearrange("b one (m p) -> m p (b one)", p=128)[mo],
                    o_sb[:])

            for kc in range(KH):
                nc.sync.dma_start(
                    hid_d.rearrange("b (k p) -> k p b", p=128)[kc],
                    h_prev[:, kc, :])

    nc.compile()
    return nc


_NC_CACHE = {}


def get_nc(n_steps=S, recur_dtype="fp32"):
    key = (n_steps, recur_dtype)
    if key not in _NC_CACHE:
        _NC_CACHE[key] = build_nc(n_steps, recur_dtype)
    return _NC_CACHE[key]


def make_in_maps(sequences, W_i2h, b_i2h, W_h2o, b_h2o):
    sequences = np.ascontiguousarray(np.asarray(sequences, dtype=np.float32))
    W_i2h = np.ascontiguousarray(np.asarray(W_i2h, dtype=np.float32))
    b_i2h = np.ascontiguousarray(np.asarray(b_i2h, dtype=np.float32))
    W_h2o = np.ascontiguousarray(np.asarray(W_h2o, dtype=np.float32))
    b_h2o = np.ascontiguousarray(np.asarray(b_h2o, dtype=np.float32))
    return [
        {
            "sequences": sequences[c * BL:(c + 1) * BL],
            "W_i2h": W_i2h,
            "b_i2h": b_i2h,
            "W_h2o": W_h2o,
            "b_h2o": b_h2o,
        }
        for c in range(N_CORES)
    ]


def kernel(sequences, W_i2h, b_i2h, W_h2o, b_h2o):
    nc = get_nc()
    in_maps = make_in_maps(sequences, W_i2h, b_i2h, W_h2o, b_h2o)
    res = run_bass_kernel_spmd(nc, in_maps, list(range(N_CORES))).results
    output = np.concatenate([res[c]["output"] for c in range(N_CORES)], axis=0)
    hidden = np.concatenate([res[c]["hidden"] for c in range(N_CORES)], axis=0)
    return output, hidden


# revision 23
# speedup vs baseline: 86.3552x; 7.5791x over previous
"""Trainium2 Bass kernel for an Elman RNN (nn_BaselineRNN).

h_t = tanh(x_t @ Wx + h_{t-1} @ Wh + b_i2h); output o = h_S @ W_h2o.T + b_h2o
for the last step only. Returns (output (B,1,O), hidden (B,H)).

Sharding: data-parallel over the batch dim across 8 NeuronCores (16 rows
per core); weights replicated. The sequential scan runs locally per core.

Per-core layout: the hidden state lives transposed ("hT": H on the 128-way
partition dim as 4 chunks of (128, 16-batch)). The per-step update
    z_m = sum_k Wh[k,m].T @ hT_k  (+ px_t + b, then tanh)
takes hT chunks in and produces hT chunks out, so the scan needs no
per-step transpose. x @ Wx is precomputed in blocks of 128 steps with
N=512 float32r matmuls (full-speed PE) into SBUF; the per-step tanh+bias
is a single ScalarE activation per chunk reading PSUM.
"""

import sys

sys.path.insert(0, "/opt/trn_rl_repo")

import numpy as np

import concourse.bacc as bacc
import concourse.bass as bass
import concourse.mybir as mybir
import concourse.tile as tile
from concourse.bass_utils import run_bass_kernel_spmd
from concourse.masks import make_identity

N_CORES = 8
B, S, I, H, O = 128, 1024, 256, 512, 256
BL = B // N_CORES  # 16 rows per core
F32 = mybir.dt.float32
F32R = mybir.dt.float32r
F16 = mybir.dt.float16

BLK = 128  # time steps per pipeline block


def _r(ap):
    return ap.bitcast(F32R)


def build_nc(n_steps=S, recur_dtype="fp32", dbg=False, repeat=1, mode="full"):
    """Build + compile the per-core Bass module.

    recur_dtype: fp32 | fp16 — dtype of Wh and h in the scan.
    mode: full | scan (skip x/px pipeline) | pe_only (also skip the
    tanh dependency: pure PE throughput probe, numerics garbage).
    repeat > 1 re-runs the whole computation (for slope timing).
    """
    assert n_steps % BLK == 0
    nblk = n_steps // BLK
    rdt = F16 if recur_dtype == "fp16" else F32
    nc = bacc.Bacc("TRN2", target_bir_lowering=False, debug=False,
                   enable_asserts=True, num_devices=N_CORES)
    if dbg:
        xt_d = nc.dram_tensor("xt_dump", [128, I // 128, BLK * BL], F32,
                              kind="ExternalOutput").ap()
        px_d = nc.dram_tensor("px_dump", [128, H // 128, BLK * BL], F32,
                              kind="ExternalOutput").ap()
        h1_d = nc.dram_tensor("h1_dump", [128, H // 128, BL], F32,
                              kind="ExternalOutput").ap()

    seq = nc.dram_tensor("sequences", [BL, S, I], F32, kind="ExternalInput").ap()
    w_i2h = nc.dram_tensor("W_i2h", [H, I + H], F32, kind="ExternalInput").ap()
    b_i2h = nc.dram_tensor("b_i2h", [H], F32, kind="ExternalInput").ap()
    w_h2o = nc.dram_tensor("W_h2o", [O, H], F32, kind="ExternalInput").ap()
    b_h2o = nc.dram_tensor("b_h2o", [O], F32, kind="ExternalInput").ap()
    out_d = nc.dram_tensor("output", [BL, 1, O], F32, kind="ExternalOutput").ap()
    hid_d = nc.dram_tensor("hidden", [BL, H], F32, kind="ExternalOutput").ap()

    KI = I // 128   # 2 contraction chunks for x@Wx
    KH = H // 128   # 4 contraction chunks for h@Wh
    MH = H // 128   # 4 output chunks of H
    MO = O // 128   # 2 output chunks of O

    with tile.TileContext(nc) as tc:
        with (
            tc.tile_pool(name="const", bufs=1) as cpool,
            tc.tile_pool(name="xnat", bufs=4) as xnat_pool,
            tc.tile_pool(name="xt", bufs=2) as xt_pool,
            tc.tile_pool(name="px", bufs=2) as px_pool,
            tc.tile_pool(name="h", bufs=2) as h_pool,
            tc.tile_pool(name="ztmp", bufs=8) as z_pool,
            tc.tile_pool(name="tp_ps", bufs=2, space="PSUM") as tp_psum,
            tc.tile_pool(name="px_ps", bufs=2, space="PSUM") as px_psum,
            tc.tile_pool(name="z_ps", bufs=4, space="PSUM") as z_psum,
        ):
            ident = cpool.tile([128, 128], F32)
            make_identity(nc, ident[:])

            # --- load weights (natural layout) and pre-transpose on PE ---
            w_nat = cpool.tile([128, 4, I + H], F32)
            nc.sync.dma_start(w_nat[:], w_i2h.rearrange("(a p) c -> p a c", p=128))
            wo_nat = cpool.tile([128, 2, H], F32)
            nc.sync.dma_start(wo_nat[:], w_h2o.rearrange("(a p) c -> p a c", p=128))

            # Wx chunks: wx[:, kc, mh*128+f] = W_i2h[mh*128+f, kc*128+p]
            # float32r: walrus requires fp32r matmul operands to be written
            # by an instruction that rounds to fp32r, so the tiles are
            # natively fp32r and the DVE copies do the rounding.
            wx = cpool.tile([128, KI, H], F32R)
            for kc in range(KI):
                for mh in range(MH):
                    ps = tp_psum.tile([128, 128], F32)
                    nc.tensor.transpose(
                        ps[:], w_nat[:, mh, kc * 128:(kc + 1) * 128], ident[:])
                    nc.vector.tensor_copy(wx[:, kc, mh * 128:(mh + 1) * 128], ps[:])

            # Wh chunks: wh[:, kc, mh*128+f] = W_i2h[mh*128+f, I + kc*128+p]
            wh = cpool.tile([128, KH, H], rdt)
            for kc in range(KH):
                for mh in range(MH):
                    ps = tp_psum.tile([128, 128], F32)
                    nc.tensor.transpose(
                        ps[:], w_nat[:, mh, I + kc * 128:I + (kc + 1) * 128], ident[:])
                    nc.vector.tensor_copy(wh[:, kc, mh * 128:(mh + 1) * 128], ps[:])

            # W_h2o.T chunks: wo[:, kc, mo*128+f] = W_h2o[mo*128+f, kc*128+p]
            wo = cpool.tile([128, KH, O], rdt)
            for kc in range(KH):
                for mo in range(MO):
                    ps = tp_psum.tile([128, 128], F32)
                    nc.tensor.transpose(
                        ps[:], wo_nat[:, mo, kc * 128:(kc + 1) * 128], ident[:])
                    nc.vector.tensor_copy(wo[:, kc, mo * 128:(mo + 1) * 128], ps[:])

            # biases: per-partition columns
            bi = cpool.tile([128, MH], F32)
            nc.sync.dma_start(bi[:], b_i2h.rearrange("(m p) -> p m", p=128))
            bo = cpool.tile([128, MO], F32)
            nc.sync.dma_start(bo[:], b_h2o.rearrange("(m p) -> p m", p=128))

            h_prev = h_pool.tile([128, KH, BL], rdt)
            nc.vector.memset(h_prev[:], 0.0)
            h_first = h_prev

            for gblk in range(nblk * repeat):
                blk = gblk % nblk
                if mode == "full":
                    # --- xT for this block: rows ordered batch-major
                    # (row = b*BLK + s), so each x load is one contiguous
                    # (128 steps, 256) tile for a single batch element.
                    xt = xt_pool.tile([128, KI, BL * BLK], F32R)
                    for bb in range(BL):
                        xn = xnat_pool.tile([128, I], F32)
                        nc.sync.dma_start(
                            xn[:], seq[bb, blk * BLK:(blk + 1) * BLK, :])
                        for kc in range(KI):
                            ps = tp_psum.tile([128, 128], F32)
                            nc.tensor.transpose(
                                ps[:], xn[:, kc * 128:(kc + 1) * 128], ident[:])
                            nc.vector.tensor_copy(
                                xt[:, kc, bb * BLK:(bb + 1) * BLK], ps[:])

                    # --- px = x @ Wx for this block (float32r, N=512) ---
                    # free order of px is (b, s): per-step slice is stride-BLK
                    px = px_pool.tile([128, MH, BL * BLK], F32)
                    px_bs = px.rearrange("p m (b s) -> p m b s", s=BLK)
                    for rg in range(BL * BLK // 512):
                        for mh in range(MH):
                            pps = px_psum.tile([128, 512], F32)
                            for kc in range(KI):
                                nc.tensor.matmul(
                                    pps[:],
                                    wx[:, kc, mh * 128:(mh + 1) * 128],
                                    xt[:, kc, rg * 512:(rg + 1) * 512],
                                    start=(kc == 0), stop=(kc == KI - 1))
                            nc.vector.tensor_copy(
                                px[:, mh, rg * 512:(rg + 1) * 512], pps[:])

                    if dbg and blk == 0:
                        nc.sync.dma_start(xt_d, xt[:].bitcast(F32))
                        nc.sync.dma_start(px_d, px[:])

                # --- the sequential scan ---
                for j in range(BLK):
                    h_new = h_pool.tile([128, KH, BL], rdt)
                    rhs_h = h_first if mode == "pe_only" else h_prev
                    for mh in range(MH):
                        zps = z_psum.tile([128, BL], F32)
                        for kc in range(KH):
                            nc.tensor.matmul(
                                zps[:],
                                wh[:, kc, mh * 128:(mh + 1) * 128],
                                rhs_h[:, kc, :],
                                start=(kc == 0), stop=(kc == KH - 1))
                        if mode == "pe_only":
                            continue
                        if mode == "full":
                            zt = z_pool.tile([128, BL], F32)
                            nc.vector.tensor_add(
                                zt[:], zps[:], px_bs[:, mh, :, j])
                        else:
                            zt = zps
                        nc.scalar.activation(
                            h_new[:, mh, :], zt[:],
                            mybir.ActivationFunctionType.Tanh,
                            bias=bi[:, mh:mh + 1])
                    if mode != "pe_only":
                        h_prev = h_new
                    if dbg and blk == 0 and j == 0:
                        nc.sync.dma_start(h1_d, h_prev[:])

            # --- output projection from the final hidden state ---
            for mo in range(MO):
                ops = z_psum.tile([128, BL], F32, tag="zps")
                for kc in range(KH):
                    nc.tensor.matmul(
                        ops[:],
                        wo[:, kc, mo * 128:(mo + 1) * 128],
                        h_prev[:, kc, :],
                        start=(kc == 0), stop=(kc == KH - 1))
                o_sb = z_pool.tile([128, BL], F32, tag="zt")
                nc.vector.tensor_scalar_add(o_sb[:], ops[:], bo[:, mo:mo + 1])
                nc.sync.dma_start(
                    out_d.rearrange("b one (m p) -> m p (b one)", p=128)[mo],
                    o_sb[:])

            if rdt != F32:
                h_fin = h_pool.tile([128, KH, BL], F32, tag="hfin")
                nc.vector.tensor_copy(h_fin[:], h_prev[:])
                h_prev = h_fin
            for kc in range(KH):
                nc.sync.dma_start(
                    hid_d.rearrange("b (k p) -> k p b", p=128)[kc],
                    h_prev[:, kc, :])

    nc.compile()
    return nc


_NC_CACHE = {}


def get_nc(n_steps=S, recur_dtype="fp32"):
    key = (n_steps, recur_dtype)
    if key not in _NC_CACHE:
        _NC_CACHE[key] = build_nc(n_steps, recur_dtype)
    return _NC_CACHE[key]


def make_in_maps(sequences, W_i2h, b_i2h, W_h2o, b_h2o):
    sequences = np.ascontiguousarray(np.asarray(sequences, dtype=np.float32))
    W_i2h = np.ascontiguousarray(np.asarray(W_i2h, dtype=np.float32))
    b_i2h = np.ascontiguousarray(np.asarray(b_i2h, dtype=np.float32))
    W_h2o = np.ascontiguousarray(np.asarray(W_h2o, dtype=np.float32))
    b_h2o = np.ascontiguousarray(np.asarray(b_h2o, dtype=np.float32))
    return [
        {
            "sequences": sequences[c * BL:(c + 1) * BL],
            "W_i2h": W_i2h,
            "b_i2h": b_i2h,
            "W_h2o": W_h2o,
            "b_h2o": b_h2o,
        }
        for c in range(N_CORES)
    ]


def kernel(sequences, W_i2h, b_i2h, W_h2o, b_h2o):
    nc = get_nc()
    in_maps = make_in_maps(sequences, W_i2h, b_i2h, W_h2o, b_h2o)
    res = run_bass_kernel_spmd(nc, in_maps, list(range(N_CORES))).results
    output = np.concatenate([res[c]["output"] for c in range(N_CORES)], axis=0)
    hidden = np.concatenate([res[c]["hidden"] for c in range(N_CORES)], axis=0)
    return output, hidden
